# revision 14
# baseline (speedup 1.0000x reference)
"""TRN2 Bass kernel for nn_BilinearInteraction — scheme A (b-on-partitions).

out[b,k] = sum_{e,f} E[b,i,e] W[k,e,f] E[b,j,f], 780 pairs (i<j), 40 fields.

Per core (batch shard 1024 = 8 b-tiles of 128):
- Stage 1 (PE): for each i, quad-groups of <=4 j-blocks:
    u[b, (slot,f)] = ET16_i(32e x 128b).T @ Wq(32e x ng*128)  -> PSUM
  (lhsT = transposed embeddings = stationary; rhs = packed W = moving)
- ScalarE evicts u -> fp16; VectorE (or GpSimd) multiplies by native-layout
  embeddings e16n[b, j, f] (slot axis aligns with contiguous j window).
- Stage 2: VectorE fp16 binary tree reduces f 32->1 per (b, slot) on two
  half-tiles per b-tile; final level adds into fp32 output rows (b, slot).
- Host gathers slots -> (8192, 780).
"""

import numpy as np

import concourse.bass as bass
import concourse.mybir as mybir
import concourse.tile as tile
from concourse import bacc
from concourse.bass_utils import run_bass_kernel_spmd

NF = 40
E = 32
NPAIR = 780
NB = NF // 4
BATCH = 8192
NCORES = 8
B_CORE = BATCH // NCORES
NBT = B_CORE // 128            # 8 b-tiles per core

# ---------------- quad tables (same as scheme B) ----------------
_quads = []
for _i in range(NF):
    for _m in range(NB):
        if 4 * _m + 3 > _i:
            _quads.append((_i, _m))
NQ = len(_quads)               # 210
NSLOT = 4 * NQ                 # 840

_pair2k = {}
_k = 0
for _i in range(NF):
    for _j in range(_i + 1, NF):
        _pair2k[(_i, _j)] = _k
        _k += 1

_res_count = [0, 0, 0, 0]
QUAD_META = []                 # (i, m, r, idx, kbase, ksize)
for _i, _m in _quads:
    _r = _i % 4
    _idx = _res_count[_r]
    _res_count[_r] += 1
    if _r < 3:
        QUAD_META.append((_i, _m, _r, _idx, 32 * _r, 32))
    else:
        QUAD_META.append((_i, _m, _r, 55 + _idx, 64, 64))
WL = max(_res_count[:3]) + _res_count[3]   # 100

SLOT_OF_K = np.full(NPAIR, -1, np.int64)
for _q, (_i, _m) in enumerate(_quads):
    for _c in range(4):
        _j = 4 * _m + _c
        if _j > _i:
            SLOT_OF_K[_pair2k[(_i, _j)]] = 4 * _q + _c
assert (SLOT_OF_K >= 0).all()

# per-i info: first quad idx, count, etc.
I_INFO = []                    # (q0, Bi, r, idx0, kbase, ksize, mmin)
_q = 0
for _i in range(NF - 1):
    q0 = _q
    while _q < NQ and _quads[_q][0] == _i:
        _q += 1
    Bi = _q - q0
    (i_, m0, r, idx0, kbase, ksize) = QUAD_META[q0]
    I_INFO.append((q0, Bi, r, idx0, kbase, ksize, m0))
assert sum(x[1] for x in I_INFO) == NQ

HALF_I = 11                    # half A: i 0..10, half B: i 11..38
SLOT_SPLIT = 4 * I_INFO[HALF_I][0]   # 392
SA, SB = SLOT_SPLIT, NSLOT - SLOT_SPLIT   # 392, 448


# ---------------- host packing ----------------
def _pack_w(W):
    Wp = np.zeros((128, WL, 128), np.float32)
    for (i, m, r, idx, kbase, ksize) in QUAD_META:
        for c in range(4):
            j = 4 * m + c
            if j > i:
                Wp[32 * r:32 * r + 32, idx, 32 * c:32 * c + 32] = W[_pair2k[(i, j)]]
    return Wp


def _pack_et(emb):
    et = emb.reshape(NCORES, B_CORE, NB, 4, E).transpose(0, 3, 4, 2, 1)
    return np.ascontiguousarray(et.reshape(NCORES, 128, NB, B_CORE))


# ---------------- bass program ----------------
_CACHED = None


def _build():
    global _CACHED
    if _CACHED is not None:
        return _CACHED

    nc = bacc.Bacc("TRN2", target_bir_lowering=False, debug=False)
    f32 = mybir.dt.float32
    f16 = mybir.dt.float16

    et16_d = nc.dram_tensor("et16", [128, NB, B_CORE], f16, kind="ExternalInput")
    wp_d = nc.dram_tensor("wp", [128, WL, 128], f16, kind="ExternalInput")
    e16n_d = nc.dram_tensor("e16n", [NBT, 128, NF, E], f16, kind="ExternalInput")
    o_d = nc.dram_tensor("o", [NBT, 128, NSLOT], f32, kind="ExternalOutput")

    with tile.TileContext(nc) as tc:
        with (
            tc.tile_pool(name="consts", bufs=1) as consts,
            tc.tile_pool(name="en", bufs=2) as en,
            tc.tile_pool(name="work", bufs=3) as work,
            tc.tile_pool(name="tpool", bufs=1) as tpool,
            tc.tile_pool(name="tree", bufs=2) as tree,
            tc.tile_pool(name="outs", bufs=2) as outs,
            tc.tile_pool(name="upsum", bufs=2, space="PSUM") as upsum,
        ):
            wp_sb = consts.tile([128, WL, 128], f16)
            for s in range(0, WL, 25):
                e = min(s + 25, WL)
                nc.sync.dma_start(out=wp_sb[:, s:e, :], in_=wp_d[:, s:e, :])
            et16_sb = consts.tile([128, NB, B_CORE], f16)
            for m in range(NB):
                nc.sync.dma_start(out=et16_sb[:, m, :], in_=et16_d[:, m, :])

            tA = tpool.tile([128, SA, E], f16, tag="tA")
            tB = tpool.tile([128, SB, E], f16, tag="tB")

            for bt in range(NBT):
                bs = bass.ts(bt, 128)
                e16n = en.tile([128, NF, E], f16, tag="e16n")
                nc.sync.dma_start(out=e16n[:], in_=e16n_d[bt, :, :, :])
                obt = outs.tile([128, NSLOT], f32, tag="obt")

                for i in range(NF - 1):
                    (q0, Bi, r, idx0, kbase, ksize, mmin) = I_INFO[i]
                    u_ps = upsum.tile([128, NB * 4, E], f32, tag="u")
                    for g0 in range(0, Bi, 4):
                        ng = min(4, Bi - g0)
                        nc.tensor.matmul(
                            u_ps[:, 4 * g0:4 * (g0 + ng), :],
                            et16_sb[kbase:kbase + ksize, i // 4, bs],
                            wp_sb[kbase:kbase + ksize, idx0 + g0:idx0 + g0 + ng, :],
                            start=True,
                            stop=True,
                        )
                    u16 = work.tile([128, NB * 4, E], f16, tag="u16")
                    nc.scalar.copy(out=u16[:, :4 * Bi, :], in_=u_ps[:, :4 * Bi, :])

                    th = tA if i < HALF_I else tB
                    s0 = 4 * q0 - (0 if i < HALF_I else SLOT_SPLIT)
                    jlo = 4 * mmin
                    if i % 4 == 3:
                        eng = nc.gpsimd
                    else:
                        eng = nc.vector
                    eng.tensor_mul(
                        th[:, s0:s0 + 4 * Bi, :],
                        u16[:, :4 * Bi, :],
                        e16n[:, jlo:jlo + 4 * Bi, :],
                    )

                    if i == HALF_I - 1 or i == NF - 2:
                        th2 = tA if i == HALF_I - 1 else tB
                        S = SA if i == HALF_I - 1 else SB
                        olo = 0 if i == HALF_I - 1 else SLOT_SPLIT
                        s1 = tree.tile([128, SB, 16], f16, tag="s1")
                        nc.vector.tensor_add(
                            s1[:, :S, :], th2[:, :, 0:16], th2[:, :, 16:32])
                        s2 = tree.tile([128, SB, 8], f16, tag="s2")
                        nc.vector.tensor_add(
                            s2[:, :S, :], s1[:, :S, 0:8], s1[:, :S, 8:16])
                        s3 = tree.tile([128, SB, 4], f16, tag="s3")
                        nc.vector.tensor_add(
                            s3[:, :S, :], s2[:, :S, 0:4], s2[:, :S, 4:8])
                        s4 = tree.tile([128, SB, 2], f16, tag="s4")
                        nc.vector.tensor_add(
                            s4[:, :S, :], s3[:, :S, 0:2], s3[:, :S, 2:4])
                        nc.vector.tensor_add(
                            obt[:, olo:olo + S],
                            s4[:, :S, 0],
                            s4[:, :S, 1],
                        )

                nc.sync.dma_start(out=o_d[bt, :, :], in_=obt[:])

    nc.compile()
    _CACHED = nc
    return nc


# ---------------- public entry ----------------
def _run(embeddings, W, **spmd_kwargs):
    embeddings = np.ascontiguousarray(np.asarray(embeddings, dtype=np.float32))
    W = np.ascontiguousarray(np.asarray(W, dtype=np.float32))

    et16 = _pack_et(embeddings).astype(np.float16)
    e16n = np.ascontiguousarray(
        embeddings.reshape(NCORES, NBT, 128, NF, E).astype(np.float16))
    wp = _pack_w(W).astype(np.float16)

    nc = _build()
    in_maps = [
        {"et16": et16[c], "wp": wp, "e16n": e16n[c]}
        for c in range(NCORES)
    ]
    res = run_bass_kernel_spmd(nc, in_maps, list(range(NCORES)), **spmd_kwargs)

    out = np.empty((BATCH, NPAIR), np.float32)
    for c in range(NCORES):
        o = res.results[c]["o"].reshape(B_CORE, NSLOT)
        out[c * B_CORE:(c + 1) * B_CORE] = o[:, SLOT_OF_K]
    return out, res


def kernel(embeddings, W):
    out, _ = _run(embeddings, W)
    return out
